# revision 11
# baseline (speedup 1.0000x reference)
"""Distributed FNO block on 8 TRN2 NeuronCores — v2.

Batch-parallel transforms + ky-sharded spectral mix (2 AllToAlls), redesigned
for tensor-engine efficiency: DFT matrices stationary with N=512 moving
operands, DMA-transposes between transform stages, y0 overlapped with the
collectives, weights streamed in large blocks, fused y0-add in the inverse.

Self-contained: shapes/sharding hardcoded, no sibling imports.
"""
import numpy as np
import ml_dtypes
from contextlib import ExitStack

import concourse.bass as bass
import concourse.bacc as bacc
import concourse.tile as tile
from concourse import mybir
from concourse.bass_utils import run_bass_kernel_spmd

B, C, H, W = 8, 128, 256, 256
M0, M1 = 32, 32
NCORES = 8
KX = np.concatenate([np.arange(32), np.arange(H - 32, H)])  # 64 kept kx modes
BF = mybir.dt.bfloat16
F32 = mybir.dt.float32
BF_NP = ml_dtypes.bfloat16


# ----------------------------------------------------------------- host consts
def _consts():
    w = np.arange(W)[:, None]
    ky = np.arange(M1)
    tw = 2 * np.pi * w * ky[None, :] / W
    FW = np.concatenate([np.cos(tw), -np.sin(tw)], axis=1)      # [256, 64]

    h = np.arange(H)[:, None]
    th = 2 * np.pi * h * KX[None, :] / H
    Ch, Sh = np.cos(th), np.sin(th)
    FH_R = np.concatenate([Ch, -Sh], axis=1)                    # [256, 128]
    FH_I = np.concatenate([Sh, Ch], axis=1)                     # [256, 128]

    thi = 2 * np.pi * np.arange(H)[None, :] * KX[:, None] / H   # [64, 256]
    GH1 = np.concatenate([np.cos(thi) / H, -np.sin(thi) / H], axis=0)  # [128, 256]
    GH2 = np.concatenate([np.sin(thi) / H, np.cos(thi) / H], axis=0)   # [128, 256]

    twi = 2 * np.pi * ky[:, None] * np.arange(W)[None, :] / W   # [32, 256]
    wt = np.where(ky == 0, 1.0, 2.0)[:, None]
    CW = np.concatenate([wt * np.cos(twi) / W, -wt * np.sin(twi) / W], axis=0)
    CW[32, :] = 0.0  # irfft drops Im(Y[ky=0])
    CW2 = np.concatenate([CW, CW], axis=0)                      # [128, 256] dup
    return (FW.astype(BF_NP), FH_R.astype(BF_NP), FH_I.astype(BF_NP),
            GH1.astype(BF_NP), GH2.astype(BF_NP), CW2.astype(BF_NP))


def _wmix_slices(w1r, w1i, w2r, w2i):
    """Per-core spectral weight slice [4ky, 64kx, 2(wr/wi), ci, co] bf16."""
    wr = np.concatenate([w1r, w2r], axis=2)  # [ci, co, 64kx, 32ky]
    wi = np.concatenate([w1i, w2i], axis=2)
    wall = np.stack([wr, wi], axis=0)        # [2, ci, co, kx, ky]
    wall = np.ascontiguousarray(wall.transpose(4, 3, 0, 1, 2)).astype(BF_NP)
    return [np.ascontiguousarray(wall[4 * k:4 * k + 4]) for k in range(NCORES)]


# ----------------------------------------------------------------- bass kernel
def _build_nc():
    nc = bacc.Bacc(num_devices=NCORES)

    xw_d = nc.declare_dram_parameter("xw", [W, 2, C, 128], BF, isOutput=False)
    xc_d = nc.declare_dram_parameter("xc", [C, H, W], BF, isOutput=False)
    wlt_d = nc.declare_dram_parameter("wlt", [C, C], BF, isOutput=False)
    fw_d = nc.declare_dram_parameter("fw", [W, 64], BF, isOutput=False)
    fhr_d = nc.declare_dram_parameter("fhr", [H, 128], BF, isOutput=False)
    fhi_d = nc.declare_dram_parameter("fhi", [H, 128], BF, isOutput=False)
    gh1_d = nc.declare_dram_parameter("gh1", [128, H], BF, isOutput=False)
    gh2_d = nc.declare_dram_parameter("gh2", [128, H], BF, isOutput=False)
    cw2_d = nc.declare_dram_parameter("cw2", [128, W], BF, isOutput=False)
    wmix_d = nc.declare_dram_parameter("wmix", [4, 64, 2, C, C], BF, isOutput=False)
    out_d = nc.declare_dram_parameter("out", [C, H, W], BF, isOutput=True)

    # internal DRAM
    y0s = nc.dram_tensor("y0s", [C, 128, W], BF)   # y0 for h >= 128 only
    send1 = nc.dram_tensor("send1", [8, 128, 4, C], BF)
    recv1 = nc.dram_tensor("recv1", [8, 128, 4, C], BF)
    xwd = nc.dram_tensor("xwd", [2, 64, C, 128], BF)
    send2 = nc.dram_tensor("send2", [8, 2, 64, 4, C], BF)
    recv2 = nc.dram_tensor("recv2", [8, 2, 64, 4, C], BF)

    rg = [list(range(NCORES))]

    with tile.TileContext(nc) as tc, ExitStack() as ctx:
        cpool = ctx.enter_context(tc.tile_pool(name="consts", bufs=1))
        xpool = ctx.enter_context(tc.tile_pool(name="xs", bufs=16))
        evpool = ctx.enter_context(tc.tile_pool(name="ev", bufs=4))
        xcpool = ctx.enter_context(tc.tile_pool(name="xcs", bufs=4))
        y0pool = ctx.enter_context(tc.tile_pool(name="y0sb", bufs=1))

        # constants into SBUF
        fw_sb = [cpool.tile([128, 64], BF, tag=f"fw{t}", name=f"fw{t}") for t in range(2)]
        fhr_sb = [cpool.tile([128, 128], BF, tag=f"fhr{t}", name=f"fhr{t}") for t in range(2)]
        fhi_sb = [cpool.tile([128, 128], BF, tag=f"fhi{t}", name=f"fhi{t}") for t in range(2)]
        for t in range(2):
            nc.sync.dma_start(fw_sb[t][:], fw_d[128 * t:128 * (t + 1), :])
            nc.sync.dma_start(fhr_sb[t][:], fhr_d[128 * t:128 * (t + 1), :])
            nc.sync.dma_start(fhi_sb[t][:], fhi_d[128 * t:128 * (t + 1), :])
        wlt_sb = cpool.tile([C, C], BF, tag="wlt")
        gh1_sb = cpool.tile([128, H], BF, tag="gh1")
        gh2_sb = cpool.tile([128, H], BF, tag="gh2")
        cw2_sb = cpool.tile([128, W], BF, tag="cw2")
        nc.sync.dma_start(wlt_sb[:], wlt_d[:])
        nc.sync.dma_start(gh1_sb[:], gh1_d[:])
        nc.sync.dma_start(gh2_sb[:], gh2_d[:])
        nc.sync.dma_start(cw2_sb[:], cw2_d[:])

        y0h = y0pool.tile([C, 128, W], BF, tag="y0h")   # y0 for h < 128

        # ------------- F-phase pool scope (closed after send1) ---------------
        fctx = ExitStack()
        fpool = fctx.enter_context(tc.tile_pool(name="fstage", bufs=1))
        xw_sb = [fpool.tile([64, C, 128], BF, tag=f"xw{t}", name=f"xw{t}") for t in range(2)]
        xwT = [fpool.tile([128, C, 64], BF, tag=f"xwT{t}", name=f"xwT{t}") for t in range(2)]
        stage1 = fpool.tile([128, 8, 4, C], BF, tag="stage1")

        # ---------------- F2': W-DFT (contract w), per (ht, c-block) ---------
        with tc.tile_pool(name="psF2", bufs=8, space="PSUM") as psF2:
            for ht in range(2):
                for cg in range(4):          # groups of 8 c-blocks
                    xts, pss = [], []
                    for i in range(8):
                        cb = 8 * cg + i
                        xt = [xpool.tile([128, 4, 128], BF, tag="xwt",
                                         name="xwt") for _ in range(2)]
                        for wh in range(2):
                            eng = nc.sync if i % 2 == 0 else nc.scalar
                            eng.dma_start(
                                xt[wh][:],
                                xw_d[128 * wh:128 * (wh + 1), ht,
                                     4 * cb:4 * cb + 4, :])
                        xts.append(xt)
                        pss.append(psF2.tile([64, 512], F32, tag="psF2",
                                             name="psF2"))
                    for wh in range(2):      # stationary-major: 2 LDW / 16 MM
                        for i in range(8):
                            nc.tensor.matmul(pss[i][:], fw_sb[wh][:],
                                             xts[i][wh][:],
                                             start=(wh == 0), stop=(wh == 1))
                    for i in range(8):
                        cb = 8 * cg + i
                        dst = xw_sb[ht][:, 4 * cb:4 * cb + 4, :]
                        src = pss[i][:].rearrange("k (c h) -> k c h", c=4)
                        if cb % 2 == 0:
                            nc.vector.tensor_copy(dst, src)
                        else:
                            nc.scalar.copy(dst, src)

        # ---------------- transpose to h-major: [64,(c h)] -> [h,c,64] -------
        # (xbar transpose needs a DRAM source; bounce through xwd)
        for ht in range(2):
            nc.sync.dma_start(xwd[ht], xw_sb[ht][:])
            nc.sync.dma_start_transpose(
                xwT[ht][:], xwd[ht].rearrange("k c h -> k (c h)"))

        # ---------------- F1': H-DFT (contract h), ky-block major ------------
        stats = [fhr_sb[0], fhi_sb[0], fhr_sb[1], fhi_sb[1]]  # (ht, ri) pairs
        combos = [(0, 0), (0, 1), (1, 0), (1, 1)]             # (ht, ri)
        with tc.tile_pool(name="psF1", bufs=8, space="PSUM") as psF1:
            for kb in range(8):
                ps = psF1.tile([128, C, 4], F32, tag="psF1")
                for si, (ht, ri) in enumerate(combos):
                    nc.tensor.matmul(
                        ps[:], stats[2 * ht + ri][:],
                        xwT[ht][:, :, 32 * ri + 4 * kb:32 * ri + 4 * kb + 4],
                        start=(si == 0), stop=(si == 3))
                dst = stage1[:, kb, :, :]
                src = ps[:].rearrange("p c k -> p k c")
                if kb % 2 == 0:
                    nc.vector.tensor_copy(dst, src)
                else:
                    nc.scalar.copy(dst, src)

        nc.sync.dma_start(send1[:].rearrange("d p k c -> p d k c"), stage1[:])
        fctx.close()
        nc.gpsimd.collective_compute(
            "AllToAll", mybir.AluOpType.bypass, replica_groups=rg,
            ins=[send1[:].opt()], outs=[recv1[:].opt()])

        # ---------------- y0 first half (h<128), overlaps A2A#1 --------------
        with tc.tile_pool(name="psY", bufs=2, space="PSUM") as psY:
            for t in range(64):
                xct = xcpool.tile([C, 2, W], BF, tag="xct")
                nc.scalar.dma_start(xct[:], xc_d[:, 2 * t:2 * t + 2, :])
                psy = psY.tile([C, 512], F32, tag="psY")
                nc.tensor.matmul(psy[:], wlt_sb[:], xct[:])
                dst = y0h[:, 2 * t:2 * t + 2, :]
                src = psy[:].rearrange("c (t w) -> c t w", t=2)
                if t % 2 == 0:
                    nc.vector.tensor_copy(dst, src)
                else:
                    nc.scalar.copy(dst, src)

            # ------------- modemix staging ----------------------------------
            mctx = ExitStack()
            mpool = mctx.enter_context(tc.tile_pool(name="mstage", bufs=1))
            mmT = mpool.tile([C, 4, 8, 128], BF, tag="mmT")
            nc.sync.dma_start_transpose(
                mmT[:].rearrange("c k b x -> c k (b x)"),
                recv1[:].rearrange("b p k c -> (b p) (k c)"))
            mm2 = mpool.tile([C, 4, 8, 2, 64], BF, tag="mm2")
            nc.vector.tensor_scalar_mul(
                mm2[:, :, :, 0, :], mmT[:, :, :, 64:128], -1.0)
            nc.scalar.copy(mm2[:, :, :, 1, :], mmT[:, :, :, 0:64])

            # ------------- modemix (ky-sharded, all batches) ----------------
            with tc.tile_pool(name="psM", bufs=6, space="PSUM") as psM, \
                 tc.tile_pool(name="wpool", bufs=3) as wpool, \
                 tc.tile_pool(name="s2pool", bufs=2) as s2pool:
                for ky in range(4):
                    # strip j = kx % 4 lives at partitions [32j, 32j+16)
                    s2t = s2pool.tile([128, 16, C], BF, tag="s2t")
                    for kxb in range(8):
                        wblk = wpool.tile([C, 8, 2, C], BF, tag="wblk")
                        nc.scalar.dma_start(wblk[:], wmix_d[ky, 8 * kxb:8 * kxb + 8])
                        for jj in range(2):      # 4 concurrent col-strips
                            psq = [psM.tile([128, C], F32, tag="psM",
                                            name="psM") for _ in range(4)]
                            for st in range(4):
                                j = 4 * jj + st
                                kx = 8 * kxb + j
                                sj = kx % 4
                                l1 = mmT[:, ky].rearrange(
                                    "c b (r x) -> c b r x", r=2)[:, :, :, kx]
                                l2 = mm2[:, ky, :, :, kx]
                                out = psq[st][32 * sj:32 * sj + 16, :]
                                nc.tensor.matmul(out, l1, wblk[:, j, 0, :],
                                                 start=True, stop=False,
                                                 tile_position=(0, 32 * sj))
                                nc.tensor.matmul(out, l2, wblk[:, j, 1, :],
                                                 start=False, stop=True,
                                                 tile_position=(0, 32 * sj))
                            for st in range(4):
                                j = 4 * jj + st
                                kx = 8 * kxb + j
                                sj = kx % 4
                                dst = s2t[32 * sj:32 * sj + 16, kx // 4, :]
                                srcp = psq[st][32 * sj:32 * sj + 16, :]
                                if st % 2 == 0:
                                    nc.vector.tensor_copy(dst, srcp)
                                else:
                                    nc.scalar.copy(dst, srcp)
                    for sj in range(4):
                        nc.sync.dma_start(
                            send2[:, :, :, ky, :].rearrange(
                                "d r (q f) c -> d r q f c", f=4)[:, :, :, sj, :]
                            .rearrange("d r q c -> (d r) q c"),
                            s2t[32 * sj:32 * sj + 16, :, :])
            mctx.close()

            nc.gpsimd.collective_compute(
                "AllToAll", mybir.AluOpType.bypass, replica_groups=rg,
                ins=[send2[:].opt()], outs=[recv2[:].opt()])

            # ------------- y0 second half (h>=128) -> DRAM, overlaps A2A#2 --
            for t in range(64):
                xct = xcpool.tile([C, 2, W], BF, tag="xct")
                nc.scalar.dma_start(xct[:], xc_d[:, 128 + 2 * t:128 + 2 * t + 2, :])
                psy = psY.tile([C, 512], F32, tag="psY")
                nc.tensor.matmul(psy[:], wlt_sb[:], xct[:])
                y0t = evpool.tile([C, 2, W], BF, tag="y0t")
                if t % 2 == 0:
                    nc.vector.tensor_copy(
                        y0t[:], psy[:].rearrange("c (t w) -> c t w", t=2))
                else:
                    nc.scalar.copy(
                        y0t[:], psy[:].rearrange("c (t w) -> c t w", t=2))
                nc.sync.dma_start(y0s[:, 2 * t:2 * t + 2, :], y0t[:])

        # ---------------- inverse ------------------------------------------
        ictx = ExitStack()
        ipool = ictx.enter_context(tc.tile_pool(name="istage", bufs=1))
        inv = ipool.tile([128, 8, 4, C], BF, tag="inv")
        nc.sync.dma_start(
            inv[:], recv2[:].rearrange("g r x k c -> (r x) g k c"))

        z_sb = ipool.tile([64, C, H], BF, tag="z")
        with tc.tile_pool(name="psI1", bufs=4, space="PSUM") as psI1:
            for co in range(C):
                psZ = psI1.tile([64, H], F32, tag="psI1")
                lhs = inv[:].rearrange("p g k c -> p (g k) c")[:, :, co]
                nc.tensor.matmul(psZ[0:32, :], lhs, gh1_sb[:],
                                 tile_position=(0, 0))
                nc.tensor.matmul(psZ[32:64, :], lhs, gh2_sb[:],
                                 tile_position=(0, 32))
                if co % 2 == 0:
                    nc.vector.tensor_copy(z_sb[0:64, co, :], psZ[:])
                else:
                    nc.scalar.copy(z_sb[0:64, co, :], psZ[:])
        with tc.tile_pool(name="psI2", bufs=4, space="PSUM") as psI2:
            for hp in range(128):
                h0, h1 = 2 * hp, 2 * hp + 1
                psO = psI2.tile([C, 2, W], F32, tag="psI2")
                nc.tensor.matmul(psO[:, 0, :], z_sb[0:64, :, h0], cw2_sb[0:64, :])
                nc.tensor.matmul(psO[:, 1, :], z_sb[0:64, :, h1], cw2_sb[0:64, :])
                outt = evpool.tile([C, 2, W], BF, tag="outt")
                if hp < 64:
                    y0sl = y0h[:, h0:h0 + 2, :]
                else:
                    y0sl = evpool.tile([C, 2, W], BF, tag="y0l")
                    nc.scalar.dma_start(y0sl[:], y0s[:, h0 - 128:h0 - 126, :])
                nc.vector.tensor_add(outt[:], psO[:], y0sl[:])
                nc.sync.dma_start(out_d[:, h0:h0 + 2, :], outt[:])
        ictx.close()

    nc.compile()
    return nc


_NC_CACHE = {}


def kernel(x, W_lin, w1r, w1i, w2r, w2i):
    x = np.asarray(x).astype(BF_NP)
    FW, FH_R, FH_I, GH1, GH2, CW2 = _consts()
    wlt = np.ascontiguousarray(np.asarray(W_lin).T).astype(BF_NP)
    wmix = _wmix_slices(np.asarray(w1r), np.asarray(w1i),
                        np.asarray(w2r), np.asarray(w2i))

    if "nc" not in _NC_CACHE:
        _NC_CACHE["nc"] = _build_nc()
    nc = _NC_CACHE["nc"]

    in_maps = []
    for k in range(NCORES):
        xk = x[k]
        xw = np.ascontiguousarray(
            xk.reshape(C, 2, 128, W).transpose(3, 1, 0, 2))
        in_maps.append({
            "xw": xw, "xc": np.ascontiguousarray(xk),
            "wlt": wlt, "fw": FW, "fhr": FH_R, "fhi": FH_I,
            "gh1": GH1, "gh2": GH2, "cw2": CW2,
            "wmix": wmix[k],
        })
    res = run_bass_kernel_spmd(nc, in_maps, list(range(NCORES)))
    out = np.stack([np.asarray(res.results[k]["out"]).astype(np.float32)
                    for k in range(NCORES)], axis=0)
    return out
